# revision 32
# baseline (speedup 1.0000x reference)
"""PhysicsAttention (structured mesh 2D) Trainium2 kernel.

Data-parallel over batch: each of the 8 NeuronCores processes one batch
element end-to-end (no collectives).

Per-core pipeline (one batch element, mesh 128x128, N=16384 pixels):
  phase A (per image row = 128 px, 4 rows per 6-row x window):
    logits : 3x3/128->512 conv as 9 matmuls, x-window stationary ->
             PIXEL-major out. slice_w and 1/temperature are folded into
             the conv weights host-side, so the PSUM holds logits/temp
             and exp() runs straight off it (no transposes, no bias).
    softmax: segmented row sums over g (DVE) + reciprocal
             -> w = e * (1/s) via a stride-0 broadcast AP
    conv_fx: same conv, same stationaries, pixel-major out
    slice_tok: accumulate st[g,c] += wT.T @ fx and norm[g] += wT.T @ 1
             (ones column appended to fx)
    w_chm  : PE-transpose w to channel-major for phase C
  phase B: normalize slice tokens, head-batched q/k/v, 64-token
    attention (no max-sub: attn logits are ~1e-3), fold out_slice with
    out_w into M[g, d]
  phase C: outT[d, n] = sum_g M[g,:].T @ w_chm[g, n]  (K=512, 4 blocks)

The conv biases and slice bias are all zero in setup_inputs(); when any
are nonzero a separate build variant adds a K=1 bias matmul per row.
"""

import numpy as np
import ml_dtypes
from contextlib import ExitStack

B = 8
HM = WM = 128
DIM = 128
HEADS = 8
DH = 64
G = 64
INNER = 512
N = HM * WM
NCORES = 8
RT = 32  # row tiles (4 image rows each)

_CACHE = {}


def _build(with_bias):
    import concourse.bass as bass
    import concourse.tile as tile
    from concourse import bacc, mybir
    from concourse.masks import make_identity

    f32 = mybir.dt.float32
    f32r = mybir.dt.float32r
    bf16 = mybir.dt.bfloat16
    AF = mybir.ActivationFunctionType
    AX = mybir.AxisListType
    ALU = mybir.AluOpType

    nc = bacc.Bacc("TRN2", target_bir_lowering=False, debug=False)
    xTp = nc.dram_tensor("xTp", [128, 130, 130], bf16, kind="ExternalInput").ap()
    wx = nc.dram_tensor("wx", [128, 9 * 512], bf16, kind="ExternalInput").ap()
    wfx = nc.dram_tensor("wfx", [128, 9 * 512], bf16, kind="ExternalInput").ap()
    wqkv = nc.dram_tensor("wqkv", [128, 192], f32, kind="ExternalInput").ap()
    owt = nc.dram_tensor("owt", [128, 1024], f32, kind="ExternalInput").ap()
    bfxp = nc.dram_tensor("bfxp", [128, 512], f32, kind="ExternalInput").ap()
    if with_bias:
        brow = nc.dram_tensor("brow", [1, 512], bf16, kind="ExternalInput").ap()
    outT = nc.dram_tensor("outT", [128, 16384], f32, kind="ExternalOutput").ap()

    with tile.TileContext(nc) as tc, ExitStack() as top:
        consts = top.enter_context(tc.tile_pool(name="consts", bufs=1))
        wchmP = top.enter_context(tc.tile_pool(name="wchmP", bufs=1))
        xwinP = top.enter_context(tc.tile_pool(name="xwin", bufs=2))

        # DMA issue order == need order: the first conv row needs only the
        # first x window + wx tap 0
        w3_0 = consts.tile([128, 3, 130], bf16)
        nc.sync.dma_start(w3_0[:], xTp[:, 0:3, :])
        wx_a = consts.tile([128, 512], bf16)
        nc.sync.dma_start(wx_a[:], wx[:, 0:512])
        w6_0 = xwinP.tile([128, 6, 130], bf16)
        nc.sync.dma_start(w6_0[:], xTp[:, 0:6, :])
        wx_b = consts.tile([128, 8 * 512], bf16)
        for c in range(4):
            nc.sync.dma_start(wx_b[:, c * 1024:(c + 1) * 1024],
                              wx[:, 512 + c * 1024: 512 + (c + 1) * 1024])
        wfx_sb = consts.tile([128, 9 * 512], bf16)
        for c in range(4):
            nc.sync.dma_start(wfx_sb[:, c * 1152:(c + 1) * 1152],
                              wfx[:, c * 1152:(c + 1) * 1152])
        wqkv_sb = consts.tile([128, 192], f32)
        nc.sync.dma_start(wqkv_sb[:], wqkv[:])
        owt_sb = consts.tile([128, 1024], f32)
        nc.sync.dma_start(owt_sb[:], owt[:])
        bfx_sb = consts.tile([128, 512], f32)
        nc.sync.dma_start(bfx_sb[:], bfxp[:])
        if with_bias:
            brow_sb = consts.tile([1, 512], bf16)
            nc.sync.dma_start(brow_sb[:], brow[:])
            ones1 = consts.tile([1, 128], bf16)
            nc.vector.memset(ones1[:], 1.0)
        idbf = consts.tile([128, 128], bf16)
        make_identity(nc, idbf[:])
        idf32 = consts.tile([128, 128], f32)
        make_identity(nc, idf32[:])
        ones_sb = consts.tile([128, 1], f32)
        nc.vector.memset(ones_sb[:], 1.0)
        M_t = []
        for p in range(4):
            M_p = consts.tile([128, 128], bf16, tag=f"M{p}")
            M_t.append(M_p)
        stn_t = []
        for p in range(4):
            stn_p = consts.tile([128, 128], f32, tag=f"stn{p}")
            stn_t.append(stn_p)

        Wchm = wchmP.tile([128, 4 * 16384], bf16)

        def wx_tap(tap):
            if tap == 0:
                return wx_a[:]
            return wx_b[:, (tap - 1) * 512: tap * 512]

        with tc.tile_pool(name="stP", bufs=1, space="PSUM") as stP:
            # two banks, each holding two (g,c)-pair regions of width 129:
            # cols [0:128) = slice_token pair, col 128 = norm (ones-column)
            psum_st0 = stP.tile([128, 258], f32, tag="st0")
            psum_st1 = stP.tile([128, 258], f32, tag="st1")
            st_banks = (psum_st0, psum_st1)

            with tc.tile_pool(name="sbA", bufs=3) as sbA, \
                 tc.tile_pool(name="psA", bufs=3, space="PSUM") as psA:
                prev = None  # (gch, wT, fx) awaiting slice_token + w_chm

                def emit_st(pr, p):
                    g_, wT_, fx_ = pr
                    nc.tensor.matmul(
                        st_banks[p // 2][:, (p % 2) * 129:(p % 2) * 129 + 129],
                        wT_[:, p * 128:(p + 1) * 128],
                        fx_[:, p, :],
                        start=(g_ == 0 and p % 2 == 0),
                        stop=(g_ == 127 and p % 2 == 1))

                def emit_pwc(pr):
                    g_, wT_, fx_ = pr
                    pwc = psA.tile([128, 512], bf16, tag="pB")
                    for q in range(4):
                        nc.tensor.transpose(pwc[:, q * 128:(q + 1) * 128],
                                            wT_[:, q * 128:(q + 1) * 128],
                                            idbf[:])
                    nc.vector.tensor_copy(
                        Wchm[:].rearrange("p (q n) -> p q n", q=4)[:, :, g_ * 128:(g_ + 1) * 128],
                        pwc[:])

                for t in range(RT):
                    if t == 0:
                        w6 = w6_0
                    else:
                        w6 = xwinP.tile([128, 6, 130], bf16)
                        nc.sync.dma_start(w6[:], xTp[:, 4 * t: 4 * t + 6, :])
                    for k in range(4):
                        gch = 4 * t + k
                        pxm = psA.tile([128, 512], f32, tag="pA")
                        if with_bias:
                            nc.tensor.matmul(pxm[:], ones1[:], brow_sb[:],
                                             start=True, stop=False)
                        # the previous row's small slice_token matmuls ride
                        # between this row's conv matmuls so their weight
                        # loads hide under the 512-wide streams
                        xw = w3_0 if gch == 0 else w6
                        xrow = (lambda r: xw[:, r, :]) if gch == 0 else \
                            (lambda r: w6[:, k + r, :])
                        for tap in range(9):
                            ky, kx = tap // 3, tap % 3
                            nc.tensor.matmul(
                                pxm[:],
                                xrow(ky)[:, kx: kx + 128],
                                wx_tap(tap),
                                start=(tap == 0 and not with_bias),
                                stop=(tap == 8))
                        if prev is not None:
                            emit_pwc(prev)
                            for p in range(4):
                                emit_st(prev, p)
                            prev = None
                        eP = sbA.tile([128, 512], bf16, tag="e", bufs=3)
                        nc.scalar.activation(eP[:], pxm[:], AF.Exp)
                        s_k = sbA.tile([128, 8], f32, tag="s", bufs=4)
                        nc.vector.reduce_sum(
                            s_k[:], eP[:].rearrange("p (h g) -> p h g", h=8),
                            axis=AX.X)
                        r_k = sbA.tile([128, 8], f32, tag="r", bufs=4)
                        nc.vector.reciprocal(r_k[:], s_k[:])
                        wT = sbA.tile([128, 512], bf16, tag="wT", bufs=3)
                        r_b = bass.AP(tensor=r_k[:].tensor, offset=r_k[:].offset,
                                      ap=[r_k[:].ap[0], [1, 8], [0, 64]])
                        nc.vector.tensor_mul(wT[:], eP[:], r_b)
                        pfx = psA.tile([128, 512], f32, tag="pA")
                        for tap in range(9):
                            ky, kx = tap // 3, tap % 3
                            nc.tensor.matmul(
                                pfx[:],
                                w6[:, k + ky, kx: kx + 128],
                                wfx_sb[:, tap * 512:(tap + 1) * 512],
                                start=(tap == 0), stop=(tap == 8))
                        fx = sbA.tile([128, 4, 129], bf16, tag="fx", bufs=3)
                        nc.scalar.activation(
                            fx[:, :, 0:128],
                            pfx[:].rearrange("p (q n) -> p q n", q=4), AF.Copy)
                        nc.vector.memset(fx[:, :, 128:129], 1.0)
                        prev = (gch, wT, fx)
                for p in range(4):
                    emit_st(prev, p)
                emit_pwc(prev)
                prev = None

            # ---- phase B part 1: normalized slice tokens (needs st PSUM) ----
            # stn = st/(norm+eps) + bfx; the exact form has the bias scaled
            # by norm/(norm+eps) (difference ~ bfx*4e-8, negligible)
            with tc.tile_pool(name="sbB1", bufs=1) as sbB1:
                ne = sbB1.tile([128, 4], f32)
                for b_ in range(2):
                    src = st_banks[b_][:]
                    nc.vector.tensor_scalar_add(
                        ne[:, 2 * b_: 2 * b_ + 2],
                        bass.AP(tensor=src.tensor, offset=src.offset + 128,
                                ap=[src.ap[0], [129, 2]]), 1e-5)
                rn = sbB1.tile([128, 4], f32)
                nc.vector.reciprocal(rn[:], ne[:])
                for p in range(4):
                    nc.vector.scalar_tensor_tensor(
                        stn_t[p][:],
                        st_banks[p // 2][:, (p % 2) * 129:(p % 2) * 129 + 128],
                        rn[:, p: p + 1],
                        bfx_sb[:, p * 128:(p + 1) * 128],
                        ALU.mult, ALU.add)

        # ---- phase B part 2: batched-head attention (st PSUM freed) ----
        with tc.tile_pool(name="sbB2", bufs=1) as sbB2, \
             tc.tile_pool(name="psB2", bufs=1, space="PSUM") as psB2:
            pstnT = psB2.tile([128, 512], f32, tag="pstnT")
            for p in range(4):
                nc.tensor.transpose(pstnT[:, p * 128:(p + 1) * 128],
                                    stn_t[p][:], idf32[:])
            stnT = sbB2.tile([128, 512], f32)
            nc.vector.tensor_copy(stnT[:], pstnT[:])
            # stnT layout: partitions (j, dh), free (p, j', g); head h=2p+j
            # lives at the diagonal block [j*64:, p*128+j*64:]
            stnT_r = stnT[:].rearrange("p (a b) -> p a b", a=4)
            pqk = psB2.tile([128, 512], f32, tag="pqk")
            pv = psB2.tile([128, 256], f32, tag="pv")
            for j in range(2):
                rhs_j = stnT_r[j * 64:(j + 1) * 64, :, j * 64:(j + 1) * 64]
                nc.tensor.matmul(pqk[j * 64:(j + 1) * 64, 0:256],
                                 wqkv_sb[j * 64:(j + 1) * 64, 0:64],
                                 rhs_j, start=True, stop=True)
                nc.tensor.matmul(pqk[j * 64:(j + 1) * 64, 256:512],
                                 wqkv_sb[j * 64:(j + 1) * 64, 64:128],
                                 rhs_j, start=True, stop=True)
            qk_sb = sbB2.tile([128, 512], f32)
            nc.vector.tensor_copy(qk_sb[:], pqk[:])
            for h in range(8):
                p_, j = h // 2, h % 2
                nc.tensor.matmul(pv[j * 64:(j + 1) * 64, p_ * 64:(p_ + 1) * 64],
                                 stnT[j * 64:(j + 1) * 64,
                                      p_ * 128 + j * 64: p_ * 128 + j * 64 + 64],
                                 wqkv_sb[j * 64:(j + 1) * 64, 128:192],
                                 start=True, stop=True)
            v_sb = sbB2.tile([128, 256], f32)
            nc.vector.tensor_copy(v_sb[:], pv[:])
            # A.T[g',g] per head, packed [ (j,g'), (p,g) ]
            pa = psB2.tile([128, 256], f32, tag="pa")
            for h in range(8):
                p_, j = h // 2, h % 2
                nc.tensor.matmul(pa[j * 64:(j + 1) * 64, p_ * 64:(p_ + 1) * 64],
                                 qk_sb[j * 64:(j + 1) * 64, 256 + p_ * 64: 256 + (p_ + 1) * 64],
                                 qk_sb[j * 64:(j + 1) * 64, p_ * 64:(p_ + 1) * 64],
                                 start=True, stop=True)
            ea = sbB2.tile([128, 256], f32)
            nc.scalar.activation(ea[:], pa[:], AF.Exp, scale=0.125)
            ps = psB2.tile([128, 4], f32, tag="ps")
            po = psB2.tile([128, 256], f32, tag="po")
            for h in range(8):
                p_, j = h // 2, h % 2
                nc.tensor.matmul(ps[j * 64:(j + 1) * 64, p_: p_ + 1],
                                 ea[j * 64:(j + 1) * 64, p_ * 64:(p_ + 1) * 64],
                                 ones_sb[j * 64:(j + 1) * 64, 0:1],
                                 start=True, stop=True)
            rs = sbB2.tile([128, 4], f32)
            nc.vector.reciprocal(rs[:], ps[:])
            for h in range(8):
                p_, j = h // 2, h % 2
                nc.tensor.matmul(po[j * 64:(j + 1) * 64, p_ * 64:(p_ + 1) * 64],
                                 v_sb[j * 64:(j + 1) * 64, p_ * 64:(p_ + 1) * 64],
                                 ea[j * 64:(j + 1) * 64, p_ * 64:(p_ + 1) * 64],
                                 start=True, stop=True)
            o_sb = sbB2.tile([128, 256], f32)
            nc.vector.tensor_copy(o_sb[:], po[:])
            pM = psB2.tile([128, 512], f32, tag="pM")
            for h in range(8):
                p_, j = h // 2, h % 2
                nc.tensor.matmul(pM[j * 64:(j + 1) * 64, p_ * 128:(p_ + 1) * 128],
                                 o_sb[j * 64:(j + 1) * 64, p_ * 64:(p_ + 1) * 64],
                                 owt_sb[j * 64:(j + 1) * 64, h * 128:(h + 1) * 128],
                                 start=True, stop=True)
            for p_ in range(4):
                nc.vector.tensor_scalar_mul(M_t[p_][:],
                                            pM[:, p_ * 128:(p_ + 1) * 128],
                                            rs[:, p_: p_ + 1])

        # ---- phase C ----
        with tc.tile_pool(name="sbC", bufs=6) as sbC, \
             tc.tile_pool(name="psC", bufs=6, space="PSUM") as psC:
            # chunks of 512 pixels; the final chunk is split in half to
            # shorten the serial copy+DMA tail
            bounds = [(i * 512, 512) for i in range(31)] + [(15872, 256), (16128, 256)]
            for i, (o0, w) in enumerate(bounds):
                po = psC.tile([128, 512], f32)
                for p in range(4):
                    nc.tensor.matmul(
                        po[:, 0:w], M_t[p][:],
                        Wchm[:, p * 16384 + o0: p * 16384 + o0 + w],
                        start=(p == 0), stop=(p == 3))
                ob = sbC.tile([128, 512], f32)
                if i % 2 == 0:
                    nc.vector.tensor_copy(ob[:, 0:w], po[:, 0:w])
                else:
                    nc.scalar.activation(ob[:, 0:w], po[:, 0:w], AF.Copy)
                nc.sync.dma_start(outT[:, o0: o0 + w], ob[:, 0:w])

    nc.compile()
    return nc


def _prep(inputs):
    x = np.asarray(inputs["x"], dtype=np.float32)
    conv_fx_w = np.asarray(inputs["conv_fx_w"], dtype=np.float32)
    conv_fx_b = np.asarray(inputs["conv_fx_b"], dtype=np.float32)
    conv_x_w = np.asarray(inputs["conv_x_w"], dtype=np.float32)
    conv_x_b = np.asarray(inputs["conv_x_b"], dtype=np.float32)
    slice_w = np.asarray(inputs["slice_w"], dtype=np.float32)
    slice_b = np.asarray(inputs["slice_b"], dtype=np.float32)
    temperature = np.asarray(inputs["temperature"], dtype=np.float32)
    wq = np.asarray(inputs["wq"], dtype=np.float32)
    wk = np.asarray(inputs["wk"], dtype=np.float32)
    wv = np.asarray(inputs["wv"], dtype=np.float32)
    out_w = np.asarray(inputs["out_w"], dtype=np.float32)

    temp = np.clip(temperature.reshape(HEADS), 0.1, 5.0)

    # fold slice_w and 1/temperature into the conv_x weights: the conv
    # then emits logits/temp directly, channel (h, g), tap-major
    Wf = np.einsum("abchd,gd->abchg",
                   conv_x_w.reshape(3, 3, DIM, HEADS, DH),
                   slice_w) / temp[None, None, None, :, None]
    wx_np = np.ascontiguousarray(
        Wf.reshape(3, 3, DIM, HEADS * G).transpose(2, 0, 1, 3)
        .reshape(128, 9 * 512)).astype(ml_dtypes.bfloat16)
    wfx_np = np.ascontiguousarray(
        conv_fx_w.transpose(2, 0, 1, 3).reshape(128, 9 * 512)).astype(ml_dtypes.bfloat16)

    bias_fold = (slice_b[None, :] + conv_x_b.reshape(HEADS, DH) @ slice_w.T) \
        / temp[:, None]  # (H, G)
    with_bias = bool(np.any(bias_fold != 0.0))
    brow_np = np.ascontiguousarray(
        bias_fold.reshape(1, HEADS * G)).astype(ml_dtypes.bfloat16)

    wqkv_half = np.concatenate([wq.T, wk.T, wv.T], axis=1).astype(np.float32)
    wqkv_np = np.vstack([wqkv_half, wqkv_half])
    owt_half = np.ascontiguousarray(
        out_w.T.reshape(8, 64, 128).transpose(1, 0, 2).reshape(64, 1024))
    owt_np = np.vstack([owt_half, owt_half])
    bfx_np = np.ascontiguousarray(
        np.tile(conv_fx_b.reshape(1, 512), (128, 1)).astype(np.float32))

    in_maps = []
    for b in range(B):
        xi = x[b].reshape(HM, WM, DIM)
        xp = np.zeros((128, 130, 130), ml_dtypes.bfloat16)
        xp[:, 1:129, 1:129] = xi.transpose(2, 0, 1).astype(ml_dtypes.bfloat16)
        m = {
            "xTp": xp, "wx": wx_np, "wfx": wfx_np,
            "wqkv": wqkv_np, "owt": owt_np, "bfxp": bfx_np,
        }
        if with_bias:
            m["brow"] = brow_np
        in_maps.append(m)
    return with_bias, in_maps


def kernel(**inputs):
    from concourse.bass_utils import run_bass_kernel_spmd

    with_bias, in_maps = _prep(inputs)
    key = ("nc", with_bias)
    if key not in _CACHE:
        _CACHE[key] = _build(with_bias)
    nc = _CACHE[key]

    res = run_bass_kernel_spmd(nc, in_maps, core_ids=list(range(NCORES)))
    out_b = np.asarray(inputs["out_b"], dtype=np.float32)
    out = np.empty((B, N, DIM), np.float32)
    for b in range(B):
        out[b] = res.results[b]["outT"].T + out_b
    return out


# revision 33
# speedup vs baseline: 1.0062x; 1.0062x over previous
"""PhysicsAttention (structured mesh 2D) Trainium2 kernel.

Data-parallel over batch: each of the 8 NeuronCores processes one batch
element end-to-end (no collectives).

Per-core pipeline (one batch element, mesh 128x128, N=16384 pixels):
  phase A (per image row = 128 px, 4 rows per 6-row x window):
    logits : 3x3/128->512 conv as 9 matmuls, x-window stationary ->
             PIXEL-major out. slice_w and 1/temperature are folded into
             the conv weights host-side, so the PSUM holds logits/temp
             and exp() runs straight off it (no transposes, no bias).
    softmax: segmented row sums over g (DVE) + reciprocal
             -> w = e * (1/s) via a stride-0 broadcast AP
    conv_fx: same conv, same stationaries, pixel-major out
    slice_tok: accumulate st[g,c] += wT.T @ fx and norm[g] += wT.T @ 1
             (ones column appended to fx)
    w_chm  : PE-transpose w to channel-major for phase C
  phase B: normalize slice tokens, head-batched q/k/v, 64-token
    attention (no max-sub: attn logits are ~1e-3), fold out_slice with
    out_w into M[g, d]
  phase C: outT[d, n] = sum_g M[g,:].T @ w_chm[g, n]  (K=512, 4 blocks)

The conv biases and slice bias are all zero in setup_inputs(); when any
are nonzero a separate build variant adds a K=1 bias matmul per row.
"""

import numpy as np
import ml_dtypes
from contextlib import ExitStack

B = 8
HM = WM = 128
DIM = 128
HEADS = 8
DH = 64
G = 64
INNER = 512
N = HM * WM
NCORES = 8
RT = 32  # row tiles (4 image rows each)

_CACHE = {}


def _build(with_bias):
    import concourse.bass as bass
    import concourse.tile as tile
    from concourse import bacc, mybir
    from concourse.masks import make_identity

    f32 = mybir.dt.float32
    f32r = mybir.dt.float32r
    bf16 = mybir.dt.bfloat16
    AF = mybir.ActivationFunctionType
    AX = mybir.AxisListType
    ALU = mybir.AluOpType

    nc = bacc.Bacc("TRN2", target_bir_lowering=False, debug=False)
    xTp = nc.dram_tensor("xTp", [128, 130, 130], bf16, kind="ExternalInput").ap()
    wx = nc.dram_tensor("wx", [128, 9 * 512], bf16, kind="ExternalInput").ap()
    wfx = nc.dram_tensor("wfx", [128, 9 * 512], bf16, kind="ExternalInput").ap()
    wqkv = nc.dram_tensor("wqkv", [128, 192], f32, kind="ExternalInput").ap()
    owt = nc.dram_tensor("owt", [128, 1024], f32, kind="ExternalInput").ap()
    bfxp = nc.dram_tensor("bfxp", [128, 512], f32, kind="ExternalInput").ap()
    if with_bias:
        brow = nc.dram_tensor("brow", [1, 512], bf16, kind="ExternalInput").ap()
    outT = nc.dram_tensor("outT", [128, 16384], f32, kind="ExternalOutput").ap()

    with tile.TileContext(nc) as tc, ExitStack() as top:
        consts = top.enter_context(tc.tile_pool(name="consts", bufs=1))
        wchmP = top.enter_context(tc.tile_pool(name="wchmP", bufs=1))
        xwinP = top.enter_context(tc.tile_pool(name="xwin", bufs=2))

        # DMA issue order == need order: the first conv row needs only the
        # first x window + wx tap 0
        w3_0 = consts.tile([128, 3, 130], bf16)
        nc.sync.dma_start(w3_0[:], xTp[:, 0:3, :])
        wx_a = consts.tile([128, 512], bf16)
        nc.sync.dma_start(wx_a[:], wx[:, 0:512])
        w6_0 = xwinP.tile([128, 6, 130], bf16)
        nc.sync.dma_start(w6_0[:], xTp[:, 0:6, :])
        wx_b = consts.tile([128, 8 * 512], bf16)
        for c in range(4):
            nc.sync.dma_start(wx_b[:, c * 1024:(c + 1) * 1024],
                              wx[:, 512 + c * 1024: 512 + (c + 1) * 1024])
        wfx_sb = consts.tile([128, 9 * 512], bf16)
        for c in range(4):
            nc.sync.dma_start(wfx_sb[:, c * 1152:(c + 1) * 1152],
                              wfx[:, c * 1152:(c + 1) * 1152])
        wqkv_sb = consts.tile([128, 192], f32)
        nc.sync.dma_start(wqkv_sb[:], wqkv[:])
        owt_sb = consts.tile([128, 1024], f32)
        nc.sync.dma_start(owt_sb[:], owt[:])
        bfx_sb = consts.tile([128, 512], f32)
        nc.sync.dma_start(bfx_sb[:], bfxp[:])
        if with_bias:
            brow_sb = consts.tile([1, 512], bf16)
            nc.sync.dma_start(brow_sb[:], brow[:])
            ones1 = consts.tile([1, 128], bf16)
            nc.vector.memset(ones1[:], 1.0)
        idbf = consts.tile([128, 128], bf16)
        make_identity(nc, idbf[:])
        idf32 = consts.tile([128, 128], f32)
        make_identity(nc, idf32[:])
        ones_sb = consts.tile([128, 1], f32)
        nc.vector.memset(ones_sb[:], 1.0)
        M_t = []
        for p in range(4):
            M_p = consts.tile([128, 128], bf16, tag=f"M{p}")
            M_t.append(M_p)
        stn_t = []
        for p in range(4):
            stn_p = consts.tile([128, 128], f32, tag=f"stn{p}")
            stn_t.append(stn_p)

        Wchm = wchmP.tile([128, 4 * 16384], bf16)

        def wx_tap(tap):
            if tap == 0:
                return wx_a[:]
            return wx_b[:, (tap - 1) * 512: tap * 512]

        with tc.tile_pool(name="stP", bufs=1, space="PSUM") as stP:
            # two banks, each holding two (g,c)-pair regions of width 129:
            # cols [0:128) = slice_token pair, col 128 = norm (ones-column)
            psum_st0 = stP.tile([128, 258], f32, tag="st0")
            psum_st1 = stP.tile([128, 258], f32, tag="st1")
            st_banks = (psum_st0, psum_st1)

            with tc.tile_pool(name="sbA", bufs=3) as sbA, \
                 tc.tile_pool(name="psA", bufs=3, space="PSUM") as psA:
                prev = None  # (gch, wT, fx) awaiting slice_token + w_chm

                def emit_st(pr, p):
                    g_, wT_, fx_ = pr
                    nc.tensor.matmul(
                        st_banks[p // 2][:, (p % 2) * 129:(p % 2) * 129 + 129],
                        wT_[:, p * 128:(p + 1) * 128],
                        fx_[:, p, :],
                        start=(g_ == 0 and p % 2 == 0),
                        stop=(g_ == 127 and p % 2 == 1))

                def emit_pwc(pr):
                    g_, wT_, fx_ = pr
                    pwc = psA.tile([128, 512], bf16, tag="pB")
                    for q in range(4):
                        nc.tensor.transpose(pwc[:, q * 128:(q + 1) * 128],
                                            wT_[:, q * 128:(q + 1) * 128],
                                            idbf[:])
                    nc.vector.tensor_copy(
                        Wchm[:].rearrange("p (q n) -> p q n", q=4)[:, :, g_ * 128:(g_ + 1) * 128],
                        pwc[:])

                for t in range(RT):
                    if t == 0:
                        w6 = w6_0
                    else:
                        w6 = xwinP.tile([128, 6, 130], bf16)
                        nc.sync.dma_start(w6[:], xTp[:, 4 * t: 4 * t + 6, :])
                    for k in range(4):
                        gch = 4 * t + k
                        pxm = psA.tile([128, 512], f32, tag="pA")
                        if with_bias:
                            nc.tensor.matmul(pxm[:], ones1[:], brow_sb[:],
                                             start=True, stop=False)
                        # the previous row's small slice_token matmuls ride
                        # between this row's conv matmuls so their weight
                        # loads hide under the 512-wide streams
                        xw = w3_0 if gch == 0 else w6
                        xrow = (lambda r: xw[:, r, :]) if gch == 0 else \
                            (lambda r: w6[:, k + r, :])
                        for tap in range(9):
                            ky, kx = tap // 3, tap % 3
                            nc.tensor.matmul(
                                pxm[:],
                                xrow(ky)[:, kx: kx + 128],
                                wx_tap(tap),
                                start=(tap == 0 and not with_bias),
                                stop=(tap == 8))
                        if prev is not None:
                            emit_pwc(prev)
                            for p in range(4):
                                emit_st(prev, p)
                            prev = None
                        eP = sbA.tile([128, 512], bf16, tag="e", bufs=3)
                        nc.scalar.activation(eP[:], pxm[:], AF.Exp)
                        s_k = sbA.tile([128, 8], f32, tag="s", bufs=4)
                        nc.vector.reduce_sum(
                            s_k[:], eP[:].rearrange("p (h g) -> p h g", h=8),
                            axis=AX.X)
                        r_k = sbA.tile([128, 8], f32, tag="r", bufs=4)
                        nc.vector.reciprocal(r_k[:], s_k[:])
                        wT = sbA.tile([128, 512], bf16, tag="wT", bufs=3)
                        r_b = bass.AP(tensor=r_k[:].tensor, offset=r_k[:].offset,
                                      ap=[r_k[:].ap[0], [1, 8], [0, 64]])
                        nc.vector.tensor_mul(wT[:], eP[:], r_b)
                        pfx = psA.tile([128, 512], f32, tag="pA")
                        for tap in range(9):
                            ky, kx = tap // 3, tap % 3
                            nc.tensor.matmul(
                                pfx[:],
                                w6[:, k + ky, kx: kx + 128],
                                wfx_sb[:, tap * 512:(tap + 1) * 512],
                                start=(tap == 0), stop=(tap == 8))
                        fx = sbA.tile([128, 4, 129], bf16, tag="fx", bufs=3)
                        nc.scalar.activation(
                            fx[:, :, 0:128],
                            pfx[:].rearrange("p (q n) -> p q n", q=4), AF.Copy)
                        nc.vector.memset(fx[:, :, 128:129], 1.0)
                        prev = (gch, wT, fx)
                for p in range(4):
                    emit_st(prev, p)
                emit_pwc(prev)
                prev = None

            # ---- phase B part 1: normalized slice tokens (needs st PSUM) ----
            # stn = st/(norm+eps) + bfx; the exact form has the bias scaled
            # by norm/(norm+eps) (difference ~ bfx*4e-8, negligible)
            with tc.tile_pool(name="sbB1", bufs=1) as sbB1:
                ne = sbB1.tile([128, 4], f32)
                for b_ in range(2):
                    src = st_banks[b_][:]
                    nc.vector.tensor_scalar_add(
                        ne[:, 2 * b_: 2 * b_ + 2],
                        bass.AP(tensor=src.tensor, offset=src.offset + 128,
                                ap=[src.ap[0], [129, 2]]), 1e-5)
                rn = sbB1.tile([128, 4], f32)
                nc.vector.reciprocal(rn[:], ne[:])
                for p in range(4):
                    nc.vector.scalar_tensor_tensor(
                        stn_t[p][:],
                        st_banks[p // 2][:, (p % 2) * 129:(p % 2) * 129 + 128],
                        rn[:, p: p + 1],
                        bfx_sb[:, p * 128:(p + 1) * 128],
                        ALU.mult, ALU.add)

        # ---- phase B part 2: batched-head attention (st PSUM freed) ----
        with tc.tile_pool(name="sbB2", bufs=1) as sbB2, \
             tc.tile_pool(name="psB2", bufs=1, space="PSUM") as psB2:
            pstnT = psB2.tile([128, 512], f32, tag="pstnT")
            for p in range(4):
                nc.tensor.transpose(pstnT[:, p * 128:(p + 1) * 128],
                                    stn_t[p][:], idf32[:])
            stnT = sbB2.tile([128, 512], f32)
            nc.vector.tensor_copy(stnT[:], pstnT[:])
            # stnT layout: partitions (j, dh), free (p, j', g); head h=2p+j
            # lives at the diagonal block [j*64:, p*128+j*64:]
            stnT_r = stnT[:].rearrange("p (a b) -> p a b", a=4)
            pqk = psB2.tile([128, 512], f32, tag="pqk")
            pv = psB2.tile([128, 256], f32, tag="pv")
            for j in range(2):
                rhs_j = stnT_r[j * 64:(j + 1) * 64, :, j * 64:(j + 1) * 64]
                nc.tensor.matmul(pqk[j * 64:(j + 1) * 64, 0:256],
                                 wqkv_sb[j * 64:(j + 1) * 64, 0:64],
                                 rhs_j, start=True, stop=True)
                nc.tensor.matmul(pqk[j * 64:(j + 1) * 64, 256:512],
                                 wqkv_sb[j * 64:(j + 1) * 64, 64:128],
                                 rhs_j, start=True, stop=True)
            qk_sb = sbB2.tile([128, 512], f32)
            nc.vector.tensor_copy(qk_sb[:], pqk[:])
            for h in range(8):
                p_, j = h // 2, h % 2
                nc.tensor.matmul(pv[j * 64:(j + 1) * 64, p_ * 64:(p_ + 1) * 64],
                                 stnT[j * 64:(j + 1) * 64,
                                      p_ * 128 + j * 64: p_ * 128 + j * 64 + 64],
                                 wqkv_sb[j * 64:(j + 1) * 64, 128:192],
                                 start=True, stop=True)
            v_sb = sbB2.tile([128, 256], f32)
            nc.vector.tensor_copy(v_sb[:], pv[:])
            # A.T[g',g] per head, packed [ (j,g'), (p,g) ]
            pa = psB2.tile([128, 256], f32, tag="pa")
            for h in range(8):
                p_, j = h // 2, h % 2
                nc.tensor.matmul(pa[j * 64:(j + 1) * 64, p_ * 64:(p_ + 1) * 64],
                                 qk_sb[j * 64:(j + 1) * 64, 256 + p_ * 64: 256 + (p_ + 1) * 64],
                                 qk_sb[j * 64:(j + 1) * 64, p_ * 64:(p_ + 1) * 64],
                                 start=True, stop=True)
            ea = sbB2.tile([128, 256], f32)
            nc.scalar.activation(ea[:], pa[:], AF.Exp, scale=0.125)
            ps = psB2.tile([128, 4], f32, tag="ps")
            po = psB2.tile([128, 256], f32, tag="po")
            for h in range(8):
                p_, j = h // 2, h % 2
                nc.tensor.matmul(ps[j * 64:(j + 1) * 64, p_: p_ + 1],
                                 ea[j * 64:(j + 1) * 64, p_ * 64:(p_ + 1) * 64],
                                 ones_sb[j * 64:(j + 1) * 64, 0:1],
                                 start=True, stop=True)
            rs = sbB2.tile([128, 4], f32)
            nc.vector.reciprocal(rs[:], ps[:])
            # two halves with separate tiles, so M (and phase C's first
            # accumulation steps) for heads 0-3 complete while heads 4-7
            # are still projecting
            pM_h = []
            o_h = []
            for half in range(2):
                pM_x = psB2.tile([128, 256], f32, tag=f"pM{half}")
                pM_h.append(pM_x)
                o_x = sbB2.tile([128, 128], f32, tag=f"o{half}")
                o_h.append(o_x)
            for half in range(2):
                for h in range(4 * half, 4 * half + 4):
                    p_, j = h // 2, h % 2
                    nc.tensor.matmul(po[j * 64:(j + 1) * 64, p_ * 64:(p_ + 1) * 64],
                                     v_sb[j * 64:(j + 1) * 64, p_ * 64:(p_ + 1) * 64],
                                     ea[j * 64:(j + 1) * 64, p_ * 64:(p_ + 1) * 64],
                                     start=True, stop=True)
                nc.vector.tensor_copy(o_h[half][:],
                                      po[:, half * 128:(half + 1) * 128])
                for h in range(4 * half, 4 * half + 4):
                    p_, j = h // 2, h % 2
                    nc.tensor.matmul(
                        pM_h[half][j * 64:(j + 1) * 64,
                                   (p_ - 2 * half) * 128:(p_ - 2 * half + 1) * 128],
                        o_h[half][j * 64:(j + 1) * 64,
                                  (p_ - 2 * half) * 64:(p_ - 2 * half + 1) * 64],
                        owt_sb[j * 64:(j + 1) * 64, h * 128:(h + 1) * 128],
                        start=True, stop=True)
                for p_ in range(2 * half, 2 * half + 2):
                    nc.vector.tensor_scalar_mul(
                        M_t[p_][:],
                        pM_h[half][:, (p_ - 2 * half) * 128:(p_ - 2 * half + 1) * 128],
                        rs[:, p_: p_ + 1])

        # ---- phase C ----
        with tc.tile_pool(name="sbC", bufs=6) as sbC, \
             tc.tile_pool(name="psC", bufs=6, space="PSUM") as psC:
            # chunks of 512 pixels; the final chunk is split in half to
            # shorten the serial copy+DMA tail
            bounds = [(i * 512, 512) for i in range(31)] + [(15872, 256), (16128, 256)]
            for i, (o0, w) in enumerate(bounds):
                po = psC.tile([128, 512], f32)
                for p in range(4):
                    nc.tensor.matmul(
                        po[:, 0:w], M_t[p][:],
                        Wchm[:, p * 16384 + o0: p * 16384 + o0 + w],
                        start=(p == 0), stop=(p == 3))
                ob = sbC.tile([128, 512], f32)
                if i % 2 == 0:
                    nc.vector.tensor_copy(ob[:, 0:w], po[:, 0:w])
                else:
                    nc.scalar.activation(ob[:, 0:w], po[:, 0:w], AF.Copy)
                nc.sync.dma_start(outT[:, o0: o0 + w], ob[:, 0:w])

    nc.compile()
    return nc


def _prep(inputs):
    x = np.asarray(inputs["x"], dtype=np.float32)
    conv_fx_w = np.asarray(inputs["conv_fx_w"], dtype=np.float32)
    conv_fx_b = np.asarray(inputs["conv_fx_b"], dtype=np.float32)
    conv_x_w = np.asarray(inputs["conv_x_w"], dtype=np.float32)
    conv_x_b = np.asarray(inputs["conv_x_b"], dtype=np.float32)
    slice_w = np.asarray(inputs["slice_w"], dtype=np.float32)
    slice_b = np.asarray(inputs["slice_b"], dtype=np.float32)
    temperature = np.asarray(inputs["temperature"], dtype=np.float32)
    wq = np.asarray(inputs["wq"], dtype=np.float32)
    wk = np.asarray(inputs["wk"], dtype=np.float32)
    wv = np.asarray(inputs["wv"], dtype=np.float32)
    out_w = np.asarray(inputs["out_w"], dtype=np.float32)

    temp = np.clip(temperature.reshape(HEADS), 0.1, 5.0)

    # fold slice_w and 1/temperature into the conv_x weights: the conv
    # then emits logits/temp directly, channel (h, g), tap-major
    Wf = np.einsum("abchd,gd->abchg",
                   conv_x_w.reshape(3, 3, DIM, HEADS, DH),
                   slice_w) / temp[None, None, None, :, None]
    wx_np = np.ascontiguousarray(
        Wf.reshape(3, 3, DIM, HEADS * G).transpose(2, 0, 1, 3)
        .reshape(128, 9 * 512)).astype(ml_dtypes.bfloat16)
    wfx_np = np.ascontiguousarray(
        conv_fx_w.transpose(2, 0, 1, 3).reshape(128, 9 * 512)).astype(ml_dtypes.bfloat16)

    bias_fold = (slice_b[None, :] + conv_x_b.reshape(HEADS, DH) @ slice_w.T) \
        / temp[:, None]  # (H, G)
    with_bias = bool(np.any(bias_fold != 0.0))
    brow_np = np.ascontiguousarray(
        bias_fold.reshape(1, HEADS * G)).astype(ml_dtypes.bfloat16)

    wqkv_half = np.concatenate([wq.T, wk.T, wv.T], axis=1).astype(np.float32)
    wqkv_np = np.vstack([wqkv_half, wqkv_half])
    owt_half = np.ascontiguousarray(
        out_w.T.reshape(8, 64, 128).transpose(1, 0, 2).reshape(64, 1024))
    owt_np = np.vstack([owt_half, owt_half])
    bfx_np = np.ascontiguousarray(
        np.tile(conv_fx_b.reshape(1, 512), (128, 1)).astype(np.float32))

    in_maps = []
    for b in range(B):
        xi = x[b].reshape(HM, WM, DIM)
        xp = np.zeros((128, 130, 130), ml_dtypes.bfloat16)
        xp[:, 1:129, 1:129] = xi.transpose(2, 0, 1).astype(ml_dtypes.bfloat16)
        m = {
            "xTp": xp, "wx": wx_np, "wfx": wfx_np,
            "wqkv": wqkv_np, "owt": owt_np, "bfxp": bfx_np,
        }
        if with_bias:
            m["brow"] = brow_np
        in_maps.append(m)
    return with_bias, in_maps


def kernel(**inputs):
    from concourse.bass_utils import run_bass_kernel_spmd

    with_bias, in_maps = _prep(inputs)
    key = ("nc", with_bias)
    if key not in _CACHE:
        _CACHE[key] = _build(with_bias)
    nc = _CACHE[key]

    res = run_bass_kernel_spmd(nc, in_maps, core_ids=list(range(NCORES)))
    out_b = np.asarray(inputs["out_b"], dtype=np.float32)
    out = np.empty((B, N, DIM), np.float32)
    for b in range(B):
        out[b] = res.results[b]["outT"].T + out_b
    return out
